# revision 4
# baseline (speedup 1.0000x reference)
"""Trainium2 Bass kernel for nn_DistanceProbeAlternative (retrieval_knn).

Computes, per batch b:
    proj = emb[b] @ W.T                      # [S, R]
    dist[i, j] = ||proj_i||^2 - 2 proj_i . proj_j + ||proj_j||^2

Sharding: data-parallel over batch B=32 across 8 cores (4 batches/core).
W is replicated. No collectives.

v7 design (DMA-roofline oriented; ~13MB HBM traffic/core @ ~420B/ns
observed single-ring rate):
  * Host lays out emb as embP16 [b, p, k, s] (p = d%128, k = d//128) so
    every input DMA moves 128 partitions x multi-KB contiguous lines.
    Batch 0 is s-split (proj h0 runs after the first 1MB); batches 1-3
    are k-split halves (partial-k accumulation).
  * Output PACKED: 8 upper-triangle block-rows per batch concatenated
    into one [128, 4608] fp16 SBUF tile, 3 contiguous DMAs per batch.
    Host unpacks + mirrors.
  * All DMA on the sync HWDGE ring; input triggers first (FIFO => input
    priority), output drains behind it.
  * PE warm-up dummy matmuls hold the HAM clock gate open until the
    input-fed matmul stream is dense.
  * Per i-tile: up to 2 dots matmuls into ONE [128, <=1024] 2-bank PSUM
    tile; ONE wide PSUM-drain op (ACT scale=-2 bias=n_i, or DVE
    tensor_scalar) + ONE wide fp16 add of rowrep (DVE or GPSIMD) --
    halves instruction/semaphore count vs 512-chunking.
  * Engine balance per batch (~5.5us each vs PE ~7.7us): ACT drains
    tiles {0,1,2,4} + projT/rowrep copies; DVE drains {3,5,6,7} + adds
    {0,1,3,5,7} + sq + ncol; GPSIMD adds {2,4,6}.
  * proj(b+1) chunks are emitted inside dots(b) so the PE never idles.
"""

import numpy as np
from contextlib import ExitStack

import concourse.bass as bass
import concourse.bacc as bacc
import concourse.tile as tile
from concourse import mybir
from concourse.bass_utils import run_bass_kernel_spmd

B, S, D, R = 32, 1024, 1024, 128
NCORES = 8
BPC = B // NCORES  # batches per core
NDT = D // 128     # 8 d-blocks
NST = S // 128     # 8 i-tiles

F32 = mybir.dt.float32
F16 = mybir.dt.float16
IDENT = mybir.ActivationFunctionType.Identity
ADD = mybir.AluOpType.add
MULT = mybir.AluOpType.mult

# packed output column offsets: tile i occupies [OFFS[i], OFFS[i]+Wi)
WIDTHS = [S - 128 * i for i in range(NST)]
OFFS = [0]
for w in WIDTHS[:-1]:
    OFFS.append(OFFS[-1] + w)
TOT = OFFS[-1] + WIDTHS[-1]  # 4608

# output drain cuts (after tile, col range)
OUT_CUTS = [(1, 0, OFFS[2]), (3, OFFS[2], OFFS[4]), (7, OFFS[4], TOT)]

N_WARM = 52  # PE warm-up dummy matmuls (N=128, ~107ns cold each)

# epilogue engine assignment per tile
PSUMOP_ENG = ['A', 'A', 'A', 'V', 'A', 'V', 'V', 'V']
ADD_ENG = ['V', 'V', 'G', 'V', 'G', 'V', 'G', 'V']


def build_nc():
    nc = bacc.Bacc("TRN2", target_bir_lowering=False, debug=False)

    embPd = nc.dram_tensor("embP16", [BPC, 128, NDT, S], F16, kind="ExternalInput")
    WTd = nc.dram_tensor("WT16", [128, D], F16, kind="ExternalInput")
    outPd = nc.dram_tensor("outP16", [BPC, 128, TOT], F16, kind="ExternalOutput")

    with tile.TileContext(nc) as tc, ExitStack() as ctx:
        constp = ctx.enter_context(tc.tile_pool(name="const", bufs=1))
        embT_p = ctx.enter_context(tc.tile_pool(name="embT", bufs=BPC))
        projT_p = ctx.enter_context(tc.tile_pool(name="projT", bufs=2))
        sq_p = ctx.enter_context(tc.tile_pool(name="sq", bufs=2))
        ncol_p = ctx.enter_context(tc.tile_pool(name="ncol", bufs=2))
        rowrep_p = ctx.enter_context(tc.tile_pool(name="rowrep", bufs=2))
        out_p = ctx.enter_context(tc.tile_pool(name="outsb", bufs=BPC))
        tmp_p = ctx.enter_context(tc.tile_pool(name="tmpsb", bufs=4))
        projps_p = ctx.enter_context(tc.tile_pool(name="projps", bufs=1, space="PSUM"))
        dotps_p = ctx.enter_context(tc.tile_pool(name="dotps", bufs=2, space="PSUM"))
        normps_p = ctx.enter_context(tc.tile_pool(name="normps", bufs=1, space="PSUM"))

        WT16 = constp.tile([128, D], F16, name="WT16")
        nc.sync.dma_start(out=WT16, in_=WTd.ap())

        ones16 = constp.tile([128, 128], F16, name="ones16")
        nc.vector.memset(ones16, 1.0)

        # ---- all input DMA triggers up front on the sync HWDGE ring ----
        # batch 0: s-halves (full k per chunk); batches 1-3: k-halves.
        embTs = []
        for b in range(BPC):
            embT = embT_p.tile([128, NDT * S], F16, name="embT")
            embTs.append(embT)
            dst = embT.rearrange("p (k s) -> p k s", k=NDT)
            src = embPd.ap()[b]
            if b == 0:
                for h in range(2):
                    sl = slice(512 * h, 512 * (h + 1))
                    nc.sync.dma_start(out=dst[:, :, sl], in_=src[:, :, sl])
            else:
                for c in range(2):
                    ks = slice(4 * c, 4 * (c + 1))
                    nc.sync.dma_start(out=dst[:, ks, :], in_=src[:, ks, :])

        # ---- PE warm-up (results never read) ----
        warm_ps = normps_p.tile([128, 128], F32, tag="np", name="warm_ps")
        for _ in range(N_WARM):
            nc.tensor.matmul(warm_ps, ones16, ones16, start=True, stop=True)

        def proj_alloc():
            projT = projT_p.tile([128, S], F16, name="projT")
            sq = sq_p.tile([128, S], F16, name="sq")
            pps = projps_p.tile([128, 1024], F32, name="projps")
            return projT, sq, pps

        def proj_finish(tiles):
            projT, sq, pps = tiles
            nc.scalar.copy(projT, pps)
            nc.vector.tensor_mul(sq, projT, projT)

        def proj_s_half(b, tiles, h):
            """Batch-0 path: full-k accumulation for s-half h."""
            embT = embTs[b]
            _, _, pps = tiles
            for k in range(NDT):
                nc.tensor.matmul(
                    pps[:, 512 * h : 512 * (h + 1)],
                    WT16[:, 128 * k : 128 * (k + 1)],
                    embT[:, S * k + 512 * h : S * k + 512 * (h + 1)],
                    start=(k == 0),
                    stop=(k == NDT - 1),
                )

        def proj_k_chunk(b, tiles, k0, k1):
            """Partial-k accumulation for k-blocks [k0, k1), both halves."""
            embT = embTs[b]
            _, _, pps = tiles
            for k in range(k0, k1):
                for h in range(2):
                    nc.tensor.matmul(
                        pps[:, 512 * h : 512 * (h + 1)],
                        WT16[:, 128 * k : 128 * (k + 1)],
                        embT[:, S * k + 512 * h : S * k + 512 * (h + 1)],
                        start=(k == 0),
                        stop=(k == NDT - 1),
                    )

        def norms_batch(sq):
            ncol_ps = normps_p.tile([128, 16], F32, tag="np", name="ncol_ps")
            for i in range(NST):
                nc.tensor.matmul(
                    ncol_ps[:, 2 * i : 2 * i + 2],
                    sq[:, 128 * i : 128 * (i + 1)],
                    ones16[:, 0:2],
                    start=True,
                    stop=True,
                )
            ncol = ncol_p.tile([128, 2 * NST], F32, name="ncol")
            nc.vector.tensor_copy(ncol, ncol_ps)

            rowrep = rowrep_p.tile([128, S], F16, name="rowrep")
            rp = normps_p.tile([128, 1024], F32, tag="np", name="rp_ps")
            for h in range(2):
                nc.tensor.matmul(
                    rp[:, 512 * h : 512 * (h + 1)],
                    ones16,
                    sq[:, 512 * h : 512 * (h + 1)],
                    start=True, stop=True,
                )
            nc.scalar.copy(rowrep, rp)
            return ncol, rowrep

        def dots_tile(b, i, outsb, projT, ncol, rowrep, no_gps=False):
            j0 = 128 * i
            Wi = WIDTHS[i]
            off = OFFS[i]
            d_ps = dotps_p.tile([128, 1024], F32, tag="dp", name="d_ps")[:, 0:Wi]
            pos = 0
            while pos < Wi:
                w = min(512, Wi - pos)
                nc.tensor.matmul(
                    d_ps[:, pos : pos + w],
                    projT[:, j0 : j0 + 128],
                    projT[:, j0 + pos : j0 + pos + w],
                    start=True,
                    stop=True,
                )
                pos += w
            o = outsb[:, off : off + Wi]
            rr = rowrep[:, j0:S]
            nb = ncol[:, 2 * i : 2 * i + 1]
            tmp = tmp_p.tile([128, 1024], F16, name="tmp")[:, 0:Wi]
            if PSUMOP_ENG[i] == 'A':
                nc.scalar.activation(tmp, d_ps, IDENT, bias=nb, scale=-2.0)
            else:
                nc.vector.tensor_scalar(tmp, d_ps, -2.0, nb, MULT, ADD)
            if ADD_ENG[i] == 'G' and not no_gps:
                nc.gpsimd.tensor_add(o, tmp, rr)
            else:
                nc.vector.tensor_add(o, tmp, rr)

        # ---- main pipeline ----
        tiles = proj_alloc()
        proj_s_half(0, tiles, 0)
        proj_s_half(0, tiles, 1)
        proj_finish(tiles)

        for b in range(BPC):
            last = b + 1 >= BPC
            projT, sq, _ = tiles
            ncol, rowrep = norms_batch(sq)
            outsb = out_p.tile([128, TOT], F16, name="outsb")
            cut = 0
            for i in range(NST):
                dots_tile(
                    b, i, outsb, projT, ncol, rowrep,
                    no_gps=(last and i >= 4),
                )
                if cut < len(OUT_CUTS) and OUT_CUTS[cut][0] == i:
                    _, c0, c1 = OUT_CUTS[cut]
                    nc.sync.dma_start(
                        out=outPd.ap()[b, :, c0:c1], in_=outsb[:, c0:c1]
                    )
                    cut += 1
                if not last:
                    if i == 1:
                        tiles_n = proj_alloc()
                        proj_k_chunk(b + 1, tiles_n, 0, 4)
                    elif i == 3:
                        proj_k_chunk(b + 1, tiles_n, 4, NDT)
                    elif i == 5:
                        proj_finish(tiles_n)
            if not last:
                tiles = tiles_n

    nc.finalize()
    return nc


_NC_CACHE = None


def _get_nc():
    global _NC_CACHE
    if _NC_CACHE is None:
        _NC_CACHE = build_nc()
    return _NC_CACHE


def _host_wt16(W):
    # WT16[p, 128k + j] = W[j, 128k + p]  (W^T in [d-part, k, r] blocks)
    Wf = np.asarray(W, dtype=np.float32)
    wt = Wf.T.reshape(NDT, 128, 128).transpose(1, 0, 2).reshape(128, D)
    return np.ascontiguousarray(wt).astype(np.float16)


def _host_embp(emb16_core):
    # embP[b, p, k, s] = emb16[b, s, 128k + p]
    return np.ascontiguousarray(
        emb16_core.reshape(BPC, S, NDT, 128).transpose(0, 3, 2, 1)
    )


def run(embeddings_batch, W, trace=False, tmpdir=None):
    nc = _get_nc()
    emb16 = np.asarray(embeddings_batch, dtype=np.float32).astype(np.float16)
    wt16 = _host_wt16(W)
    in_maps = [
        {
            "embP16": _host_embp(emb16[c * BPC : (c + 1) * BPC]),
            "WT16": wt16,
        }
        for c in range(NCORES)
    ]
    res = run_bass_kernel_spmd(
        nc, in_maps, core_ids=list(range(NCORES)), trace=trace, tmpdir=tmpdir
    )
    # unpack: outP16 [BPC, 128, TOT] -> dist blocks j >= i; mirror the rest
    full = np.empty((B, S, S), dtype=np.float16)
    for c in range(NCORES):
        P = res.results[c]["outP16"]
        for b in range(BPC):
            g = c * BPC + b
            for i in range(NST):
                full[g, 128 * i : 128 * (i + 1), 128 * i : S] = P[
                    b, :, OFFS[i] : OFFS[i] + WIDTHS[i]
                ]
    NB = NST
    M = full.reshape(B, NB, 128, NB, 128)
    iu = np.triu_indices(NB, 1)
    M[:, iu[1], :, iu[0], :] = M[:, iu[0], :, iu[1], :].swapaxes(-1, -2)
    return full.astype(np.float32), res


def kernel(embeddings_batch, W):
    full, _ = run(embeddings_batch, W, trace=False)
    return full


# revision 5
# speedup vs baseline: 1.1390x; 1.1390x over previous
"""Trainium2 Bass kernel for nn_DistanceProbeAlternative (retrieval_knn).

Computes, per batch b:
    proj = emb[b] @ W.T                      # [S, R]
    dist[i, j] = ||proj_i||^2 - 2 proj_i . proj_j + ||proj_j||^2

Sharding: data-parallel over batch B=32 across 8 cores (4 batches/core).
W is replicated. No collectives.

v8 design:
  * Host lays out emb as embP16 [b, p, k, s] (p = d%128, k = d//128):
    input DMAs move 128 partitions x multi-KB contiguous lines on the
    sync HWDGE ring at ~420 B/ns. Batch 0 s-halved (early PE start),
    batches 1-3 k-halved (partial-k accumulation interleaved into the
    previous batch's dots).
  * Output PACKED [128, 4608] fp16 per batch, 3-4 contiguous DMAs as
    tile groups complete; host unpacks + mirrors. All DMA on one ring:
    input first (FIFO priority), output drains behind.
  * PE warm-up dummies hold the HAM clock gate at 2.4GHz until the
    input-fed matmul stream is dense.
  * Per i-tile: dots matmuls in <=512 chunks into a 2-bank PSUM tile
    (dotps pool bufs=5 -- deep recycling, no PE stalls); per-chunk
    PSUM drain (-2*d + n_i) split ACT/DVE; ONE wide fp16 add of
    rowrep (n_j) per tile on DVE (tiles 0-3) / GPSIMD (tiles 4-7).
    sq runs on GPSIMD (SBUF-only op, frees DVE).
  * Engine budget/batch ~6us each on ACT/DVE/GPS vs PE ~7.4us.
"""

import numpy as np
from contextlib import ExitStack

import concourse.bass as bass
import concourse.bacc as bacc
import concourse.tile as tile
from concourse import mybir
from concourse.bass_utils import run_bass_kernel_spmd

B, S, D, R = 32, 1024, 1024, 128
NCORES = 8
BPC = B // NCORES  # batches per core
NDT = D // 128     # 8 d-blocks
NST = S // 128     # 8 i-tiles

F32 = mybir.dt.float32
F16 = mybir.dt.float16
IDENT = mybir.ActivationFunctionType.Identity
ADD = mybir.AluOpType.add
MULT = mybir.AluOpType.mult

WIDTHS = [S - 128 * i for i in range(NST)]
OFFS = [0]
for w in WIDTHS[:-1]:
    OFFS.append(OFFS[-1] + w)
TOT = OFFS[-1] + WIDTHS[-1]  # 4608

# output drain cuts (after tile -> col range)
OUT_CUTS = [(1, 0, OFFS[2]), (3, OFFS[2], OFFS[4]), (7, OFFS[4], TOT)]
OUT_CUTS_LAST = [
    (1, 0, OFFS[2]), (3, OFFS[2], OFFS[4]),
    (5, OFFS[4], OFFS[6]), (7, OFFS[6], TOT),
]

N_WARM = 52  # PE warm-up dummy matmuls

# drain engine per tile-chunk index (12 chunks: t0h0,t0h1,t1h0,t1h1,
# t2h0,t2h1,t3h0,t3h1,t4,t5,t6,t7): ACT for tiles 0-2, DVE for 3-7
DRAIN_ACT_TILES = (0, 1, 2)
# wide rowrep-add engine per tile: DVE tiles 0-3, GPS tiles 4-7
ADD_GPS_TILES = (4, 5, 6, 7)


def build_nc():
    nc = bacc.Bacc("TRN2", target_bir_lowering=False, debug=False)

    embPd = nc.dram_tensor("embP16", [BPC, 128, NDT, S], F16, kind="ExternalInput")
    WTd = nc.dram_tensor("WT16", [128, D], F16, kind="ExternalInput")
    outPd = nc.dram_tensor("outP16", [BPC, 128, TOT], F16, kind="ExternalOutput")

    with tile.TileContext(nc) as tc, ExitStack() as ctx:
        constp = ctx.enter_context(tc.tile_pool(name="const", bufs=1))
        embT_p = ctx.enter_context(tc.tile_pool(name="embT", bufs=BPC))
        projT_p = ctx.enter_context(tc.tile_pool(name="projT", bufs=2))
        sq_p = ctx.enter_context(tc.tile_pool(name="sq", bufs=2))
        ncol_p = ctx.enter_context(tc.tile_pool(name="ncol", bufs=2))
        rowrep_p = ctx.enter_context(tc.tile_pool(name="rowrep", bufs=2))
        out_p = ctx.enter_context(tc.tile_pool(name="outsb", bufs=BPC))
        tmp_p = ctx.enter_context(tc.tile_pool(name="tmpsb", bufs=4))
        projps_p = ctx.enter_context(tc.tile_pool(name="projps", bufs=1, space="PSUM"))
        dotps_p = ctx.enter_context(tc.tile_pool(name="dotps", bufs=5, space="PSUM"))
        normps_p = ctx.enter_context(tc.tile_pool(name="normps", bufs=1, space="PSUM"))

        WT16 = constp.tile([128, D], F16, name="WT16")
        nc.sync.dma_start(out=WT16, in_=WTd.ap())

        ones16 = constp.tile([128, 128], F16, name="ones16")
        nc.vector.memset(ones16, 1.0)

        # ---- all input DMA triggers up front on the sync HWDGE ring ----
        embTs = []
        for b in range(BPC):
            embT = embT_p.tile([128, NDT * S], F16, name="embT")
            embTs.append(embT)
            dst = embT.rearrange("p (k s) -> p k s", k=NDT)
            src = embPd.ap()[b]
            if b == 0:
                for h in range(2):
                    sl = slice(512 * h, 512 * (h + 1))
                    nc.sync.dma_start(out=dst[:, :, sl], in_=src[:, :, sl])
            else:
                for c in range(2):
                    ks = slice(4 * c, 4 * (c + 1))
                    nc.sync.dma_start(out=dst[:, ks, :], in_=src[:, ks, :])

        # ---- PE warm-up (results never read) ----
        warm_ps = normps_p.tile([128, 128], F32, tag="np", name="warm_ps")
        for _ in range(N_WARM):
            nc.tensor.matmul(warm_ps, ones16, ones16, start=True, stop=True)

        def proj_alloc():
            projT = projT_p.tile([128, S], F16, name="projT")
            sq = sq_p.tile([128, S], F16, name="sq")
            pps = projps_p.tile([128, 1024], F32, name="projps")
            return projT, sq, pps

        def proj_finish(tiles):
            projT, sq, pps = tiles
            nc.scalar.copy(projT, pps)          # ACT, 1024 wide
            nc.gpsimd.tensor_mul(sq, projT, projT)  # GPS, SBUF-only

        def proj_s_half(b, tiles, h):
            embT = embTs[b]
            _, _, pps = tiles
            for k in range(NDT):
                nc.tensor.matmul(
                    pps[:, 512 * h : 512 * (h + 1)],
                    WT16[:, 128 * k : 128 * (k + 1)],
                    embT[:, S * k + 512 * h : S * k + 512 * (h + 1)],
                    start=(k == 0),
                    stop=(k == NDT - 1),
                )

        def proj_k_chunk(b, tiles, k0, k1):
            embT = embTs[b]
            _, _, pps = tiles
            for k in range(k0, k1):
                for h in range(2):
                    nc.tensor.matmul(
                        pps[:, 512 * h : 512 * (h + 1)],
                        WT16[:, 128 * k : 128 * (k + 1)],
                        embT[:, S * k + 512 * h : S * k + 512 * (h + 1)],
                        start=(k == 0),
                        stop=(k == NDT - 1),
                    )

        def norms_batch(sq):
            ncol_ps = normps_p.tile([128, 16], F32, tag="np", name="ncol_ps")
            for i in range(NST):
                nc.tensor.matmul(
                    ncol_ps[:, 2 * i : 2 * i + 2],
                    sq[:, 128 * i : 128 * (i + 1)],
                    ones16[:, 0:2],
                    start=True,
                    stop=True,
                )
            ncol = ncol_p.tile([128, 2 * NST], F32, name="ncol")
            nc.vector.tensor_copy(ncol, ncol_ps)

            rowrep = rowrep_p.tile([128, S], F16, name="rowrep")
            for h in range(2):
                rp = normps_p.tile([128, 512], F32, tag="np", name="rp_ps")
                nc.tensor.matmul(
                    rp, ones16, sq[:, 512 * h : 512 * (h + 1)],
                    start=True, stop=True,
                )
                nc.scalar.copy(rowrep[:, 512 * h : 512 * (h + 1)], rp)
            return ncol, rowrep

        def dots_tile(b, i, outsb, projT, ncol, rowrep, last):
            j0 = 128 * i
            Wi = WIDTHS[i]
            off = OFFS[i]
            nb = ncol[:, 2 * i : 2 * i + 1]
            tmp = tmp_p.tile([128, 1024], F16, name="tmp")[:, 0:Wi]
            pos = 0
            while pos < Wi:
                w = min(512, Wi - pos)
                d_ps = dotps_p.tile([128, w], F32, tag="dp", name="d_ps")
                nc.tensor.matmul(
                    d_ps,
                    projT[:, j0 : j0 + 128],
                    projT[:, j0 + pos : j0 + pos + w],
                    start=True,
                    stop=True,
                )
                tc_ = tmp[:, pos : pos + w]
                if i in DRAIN_ACT_TILES:
                    nc.scalar.activation(tc_, d_ps, IDENT, bias=nb, scale=-2.0)
                else:
                    nc.vector.tensor_scalar(tc_, d_ps, -2.0, nb, MULT, ADD)
                pos += w
            o = outsb[:, off : off + Wi]
            rr = rowrep[:, j0:S]
            if i in ADD_GPS_TILES and not (last and i >= 6):
                nc.gpsimd.tensor_add(o, tmp, rr)
            else:
                nc.vector.tensor_add(o, tmp, rr)

        # ---- main pipeline ----
        tiles = proj_alloc()
        proj_s_half(0, tiles, 0)
        proj_s_half(0, tiles, 1)
        proj_finish(tiles)

        for b in range(BPC):
            last = b + 1 >= BPC
            projT, sq, _ = tiles
            ncol, rowrep = norms_batch(sq)
            outsb = out_p.tile([128, TOT], F16, name="outsb")
            cuts = OUT_CUTS_LAST if last else OUT_CUTS
            cut = 0
            for i in range(NST):
                dots_tile(b, i, outsb, projT, ncol, rowrep, last)
                if cut < len(cuts) and cuts[cut][0] == i:
                    _, c0, c1 = cuts[cut]
                    nc.sync.dma_start(
                        out=outPd.ap()[b, :, c0:c1], in_=outsb[:, c0:c1]
                    )
                    cut += 1
                if not last:
                    if i == 1:
                        tiles_n = proj_alloc()
                        proj_k_chunk(b + 1, tiles_n, 0, 4)
                    elif i == 3:
                        proj_k_chunk(b + 1, tiles_n, 4, NDT)
                    elif i == 5:
                        proj_finish(tiles_n)
            if not last:
                tiles = tiles_n

    nc.finalize()
    return nc


_NC_CACHE = None


def _get_nc():
    global _NC_CACHE
    if _NC_CACHE is None:
        _NC_CACHE = build_nc()
    return _NC_CACHE


def _host_wt16(W):
    # WT16[p, 128k + j] = W[j, 128k + p]
    Wf = np.asarray(W, dtype=np.float32)
    wt = Wf.T.reshape(NDT, 128, 128).transpose(1, 0, 2).reshape(128, D)
    return np.ascontiguousarray(wt).astype(np.float16)


def _host_embp(emb16_core):
    # embP[b, p, k, s] = emb16[b, s, 128k + p]
    return np.ascontiguousarray(
        emb16_core.reshape(BPC, S, NDT, 128).transpose(0, 3, 2, 1)
    )


def run(embeddings_batch, W, trace=False, tmpdir=None):
    nc = _get_nc()
    emb16 = np.asarray(embeddings_batch, dtype=np.float32).astype(np.float16)
    wt16 = _host_wt16(W)
    in_maps = [
        {
            "embP16": _host_embp(emb16[c * BPC : (c + 1) * BPC]),
            "WT16": wt16,
        }
        for c in range(NCORES)
    ]
    res = run_bass_kernel_spmd(
        nc, in_maps, core_ids=list(range(NCORES)), trace=trace, tmpdir=tmpdir
    )
    full = np.empty((B, S, S), dtype=np.float16)
    for c in range(NCORES):
        P = res.results[c]["outP16"]
        for b in range(BPC):
            g = c * BPC + b
            for i in range(NST):
                full[g, 128 * i : 128 * (i + 1), 128 * i : S] = P[
                    b, :, OFFS[i] : OFFS[i] + WIDTHS[i]
                ]
    NB = NST
    M = full.reshape(B, NB, 128, NB, 128)
    iu = np.triu_indices(NB, 1)
    M[:, iu[1], :, iu[0], :] = M[:, iu[0], :, iu[1], :].swapaxes(-1, -2)
    return full.astype(np.float32), res


def kernel(embeddings_batch, W):
    full, _ = run(embeddings_batch, W, trace=False)
    return full


# revision 6
# speedup vs baseline: 1.1571x; 1.0159x over previous
"""Trainium2 Bass kernel for nn_DistanceProbeAlternative (retrieval_knn).

Computes, per batch b:
    proj = emb[b] @ W.T                      # [S, R]
    dist[i, j] = ||proj_i||^2 - 2 proj_i . proj_j + ||proj_j||^2

Sharding: data-parallel over batch B=32 across 8 cores (4 batches/core).
W is replicated. No collectives.

v9 design:
  * Host lays out emb as embP16 [b, p, k, s]: input DMAs move 128
    partitions x multi-KB contiguous lines on the sync HWDGE ring at
    ~420 B/ns. Batch 0 s-halved (early PE start + per-half finish),
    batches 1-3 k-halved, interleaved into the previous batch's dots.
  * Output PACKED [128, 4608] fp16 per batch, 3-4 contiguous DMAs;
    host unpacks + mirrors. All DMA on one ring, input first.
  * PE warm-up dummies hold the HAM clock at 2.4GHz until the real
    matmul stream is dense.
  * sq is fp8e4 (norm matmuls all-fp8; dist err contribution ~3e-3,
    tolerance 2e-2): cheap DVE op off the critical path, fp8 FWL
    LDWEIGHTS for the 8 ncol matmuls.
  * Epilogue per i-tile: <=512 dots chunks into dotps (bufs=5), drains
    (-2*d + n_i) on ACT (tiles 0-2) / DVE tensor_scalar (tiles 3-7),
    ONE wide fp16 rowrep add per tile on DVE (tiles 0-3) / GPSIMD
    (tiles 4-7; last batch tiles 6-7 on DVE for a short tail).
"""

import numpy as np
from contextlib import ExitStack

import concourse.bass as bass
import concourse.bacc as bacc
import concourse.tile as tile
from concourse import mybir
from concourse.bass_utils import run_bass_kernel_spmd

B, S, D, R = 32, 1024, 1024, 128
NCORES = 8
BPC = B // NCORES
NDT = D // 128
NST = S // 128

F32 = mybir.dt.float32
F16 = mybir.dt.float16
F8 = mybir.dt.float8e4
IDENT = mybir.ActivationFunctionType.Identity
ADD = mybir.AluOpType.add
MULT = mybir.AluOpType.mult

WIDTHS = [S - 128 * i for i in range(NST)]
OFFS = [0]
for w in WIDTHS[:-1]:
    OFFS.append(OFFS[-1] + w)
TOT = OFFS[-1] + WIDTHS[-1]  # 4608

OUT_CUTS = [(1, 0, OFFS[2]), (3, OFFS[2], OFFS[4]), (7, OFFS[4], TOT)]
OUT_CUTS_LAST = [
    (1, 0, OFFS[2]), (3, OFFS[2], OFFS[4]),
    (5, OFFS[4], OFFS[6]), (7, OFFS[6], TOT),
]

N_WARM = 52

DRAIN_ACT_TILES = (0, 1, 2)   # drains on ACT; rest on DVE
ADD_GPS_TILES = (4, 5, 6, 7)  # wide adds on GPSIMD; rest on DVE


def build_nc():
    nc = bacc.Bacc("TRN2", target_bir_lowering=False, debug=False)

    embPd = nc.dram_tensor("embP16", [BPC, 128, NDT, S], F16, kind="ExternalInput")
    WTd = nc.dram_tensor("WT16", [128, D], F16, kind="ExternalInput")
    outPd = nc.dram_tensor("outP16", [BPC, 128, TOT], F16, kind="ExternalOutput")

    with tile.TileContext(nc) as tc, ExitStack() as ctx:
        constp = ctx.enter_context(tc.tile_pool(name="const", bufs=1))
        embT_p = ctx.enter_context(tc.tile_pool(name="embT", bufs=BPC))
        projT_p = ctx.enter_context(tc.tile_pool(name="projT", bufs=2))
        sq_p = ctx.enter_context(tc.tile_pool(name="sq", bufs=2))
        ncol_p = ctx.enter_context(tc.tile_pool(name="ncol", bufs=2))
        rowrep_p = ctx.enter_context(tc.tile_pool(name="rowrep", bufs=2))
        out_p = ctx.enter_context(tc.tile_pool(name="outsb", bufs=BPC))
        tmp_p = ctx.enter_context(tc.tile_pool(name="tmpsb", bufs=4))
        projps_p = ctx.enter_context(tc.tile_pool(name="projps", bufs=1, space="PSUM"))
        dotps_p = ctx.enter_context(tc.tile_pool(name="dotps", bufs=5, space="PSUM"))
        normps_p = ctx.enter_context(tc.tile_pool(name="normps", bufs=1, space="PSUM"))

        WT16 = constp.tile([128, D], F16, name="WT16")
        nc.sync.dma_start(out=WT16, in_=WTd.ap())

        ones16 = constp.tile([128, 128], F16, name="ones16")
        nc.vector.memset(ones16, 1.0)
        ones8 = constp.tile([128, 128], F8, name="ones8")
        nc.vector.memset(ones8, 1.0)

        embTs = []
        for b in range(BPC):
            embT = embT_p.tile([128, NDT * S], F16, name="embT")
            embTs.append(embT)
            dst = embT.rearrange("p (k s) -> p k s", k=NDT)
            src = embPd.ap()[b]
            if b == 0:
                for h in range(2):
                    sl = slice(512 * h, 512 * (h + 1))
                    nc.sync.dma_start(out=dst[:, :, sl], in_=src[:, :, sl])
            else:
                for c in range(2):
                    ks = slice(4 * c, 4 * (c + 1))
                    nc.sync.dma_start(out=dst[:, ks, :], in_=src[:, ks, :])

        warm_ps = normps_p.tile([128, 128], F32, tag="np", name="warm_ps")
        for _ in range(N_WARM):
            nc.tensor.matmul(warm_ps, ones16, ones16, start=True, stop=True)

        def proj_alloc():
            projT = projT_p.tile([128, S], F16, name="projT")
            sq = sq_p.tile([128, S], F8, name="sq")
            pps = projps_p.tile([128, 1024], F32, name="projps")
            return projT, sq, pps

        def proj_finish(tiles, h=None):
            projT, sq, pps = tiles
            sl = slice(0, S) if h is None else slice(512 * h, 512 * (h + 1))
            nc.scalar.copy(projT[:, sl], pps[:, sl])
            nc.vector.tensor_mul(sq[:, sl], projT[:, sl], projT[:, sl])

        def proj_s_half(b, tiles, h):
            embT = embTs[b]
            _, _, pps = tiles
            for k in range(NDT):
                nc.tensor.matmul(
                    pps[:, 512 * h : 512 * (h + 1)],
                    WT16[:, 128 * k : 128 * (k + 1)],
                    embT[:, S * k + 512 * h : S * k + 512 * (h + 1)],
                    start=(k == 0),
                    stop=(k == NDT - 1),
                )

        def proj_k_chunk(b, tiles, k0, k1):
            embT = embTs[b]
            _, _, pps = tiles
            for k in range(k0, k1):
                for h in range(2):
                    nc.tensor.matmul(
                        pps[:, 512 * h : 512 * (h + 1)],
                        WT16[:, 128 * k : 128 * (k + 1)],
                        embT[:, S * k + 512 * h : S * k + 512 * (h + 1)],
                        start=(k == 0),
                        stop=(k == NDT - 1),
                    )

        def norms_batch(sq):
            ncol_ps = normps_p.tile([128, 16], F32, tag="np", name="ncol_ps")
            for i in range(NST):
                nc.tensor.matmul(
                    ncol_ps[:, 2 * i : 2 * i + 2],
                    sq[:, 128 * i : 128 * (i + 1)],
                    ones8[:, 0:2],
                    start=True,
                    stop=True,
                )
            ncol = ncol_p.tile([128, 2 * NST], F32, name="ncol")
            nc.vector.tensor_copy(ncol, ncol_ps)

            rowrep = rowrep_p.tile([128, S], F16, name="rowrep")
            for h in range(2):
                rp = normps_p.tile([128, 512], F32, tag="np", name="rp_ps")
                nc.tensor.matmul(
                    rp, ones8, sq[:, 512 * h : 512 * (h + 1)],
                    start=True, stop=True,
                )
                nc.scalar.copy(rowrep[:, 512 * h : 512 * (h + 1)], rp)
            return ncol, rowrep

        def dots_tile(b, i, outsb, projT, ncol, rowrep, last):
            j0 = 128 * i
            Wi = WIDTHS[i]
            off = OFFS[i]
            nb = ncol[:, 2 * i : 2 * i + 1]
            tmp = tmp_p.tile([128, 1024], F16, name="tmp")[:, 0:Wi]
            pos = 0
            while pos < Wi:
                w = min(512, Wi - pos)
                d_ps = dotps_p.tile([128, w], F32, tag="dp", name="d_ps")
                nc.tensor.matmul(
                    d_ps,
                    projT[:, j0 : j0 + 128],
                    projT[:, j0 + pos : j0 + pos + w],
                    start=True,
                    stop=True,
                )
                tc_ = tmp[:, pos : pos + w]
                if i in DRAIN_ACT_TILES:
                    nc.scalar.activation(tc_, d_ps, IDENT, bias=nb, scale=-2.0)
                else:
                    nc.vector.tensor_scalar(tc_, d_ps, -2.0, nb, MULT, ADD)
                pos += w
            o = outsb[:, off : off + Wi]
            rr = rowrep[:, j0:S]
            if i in ADD_GPS_TILES and not (last and i >= 6):
                nc.gpsimd.tensor_add(o, tmp, rr)
            else:
                nc.vector.tensor_add(o, tmp, rr)

        # ---- main pipeline ----
        tiles = proj_alloc()
        proj_s_half(0, tiles, 0)
        proj_finish(tiles, 0)
        proj_s_half(0, tiles, 1)
        proj_finish(tiles, 1)

        for b in range(BPC):
            last = b + 1 >= BPC
            projT, sq, _ = tiles
            ncol, rowrep = norms_batch(sq)
            outsb = out_p.tile([128, TOT], F16, name="outsb")
            cuts = OUT_CUTS_LAST if last else OUT_CUTS
            cut = 0
            for i in range(NST):
                dots_tile(b, i, outsb, projT, ncol, rowrep, last)
                if cut < len(cuts) and cuts[cut][0] == i:
                    _, c0, c1 = cuts[cut]
                    nc.sync.dma_start(
                        out=outPd.ap()[b, :, c0:c1], in_=outsb[:, c0:c1]
                    )
                    cut += 1
                if not last:
                    if i == 1:
                        tiles_n = proj_alloc()
                        proj_k_chunk(b + 1, tiles_n, 0, 4)
                    elif i == 3:
                        proj_k_chunk(b + 1, tiles_n, 4, NDT)
                        proj_finish(tiles_n)
            if not last:
                tiles = tiles_n

    nc.finalize()
    return nc


_NC_CACHE = None


def _get_nc():
    global _NC_CACHE
    if _NC_CACHE is None:
        _NC_CACHE = build_nc()
    return _NC_CACHE


def _host_wt16(W):
    Wf = np.asarray(W, dtype=np.float32)
    wt = Wf.T.reshape(NDT, 128, 128).transpose(1, 0, 2).reshape(128, D)
    return np.ascontiguousarray(wt).astype(np.float16)


def _host_embp(emb16_core):
    return np.ascontiguousarray(
        emb16_core.reshape(BPC, S, NDT, 128).transpose(0, 3, 2, 1)
    )


def run(embeddings_batch, W, trace=False, tmpdir=None):
    nc = _get_nc()
    emb16 = np.asarray(embeddings_batch, dtype=np.float32).astype(np.float16)
    wt16 = _host_wt16(W)
    in_maps = [
        {
            "embP16": _host_embp(emb16[c * BPC : (c + 1) * BPC]),
            "WT16": wt16,
        }
        for c in range(NCORES)
    ]
    res = run_bass_kernel_spmd(
        nc, in_maps, core_ids=list(range(NCORES)), trace=trace, tmpdir=tmpdir
    )
    full = np.empty((B, S, S), dtype=np.float16)
    for c in range(NCORES):
        P = res.results[c]["outP16"]
        for b in range(BPC):
            g = c * BPC + b
            for i in range(NST):
                full[g, 128 * i : 128 * (i + 1), 128 * i : S] = P[
                    b, :, OFFS[i] : OFFS[i] + WIDTHS[i]
                ]
    NB = NST
    M = full.reshape(B, NB, 128, NB, 128)
    iu = np.triu_indices(NB, 1)
    M[:, iu[1], :, iu[0], :] = M[:, iu[0], :, iu[1], :].swapaxes(-1, -2)
    return full.astype(np.float32), res


def kernel(embeddings_batch, W):
    full, _ = run(embeddings_batch, W, trace=False)
    return full


# revision 7
# speedup vs baseline: 1.3473x; 1.1644x over previous
"""Trainium2 Bass kernel for nn_DistanceProbeAlternative (retrieval_knn).

Computes, per batch b:
    proj = emb[b] @ W.T                      # [S, R]
    dist[i, j] = ||proj_i||^2 - 2 proj_i . proj_j + ||proj_j||^2

Sharding: data-parallel over batch B=32 across 8 cores (4 batches/core).
W is replicated. No collectives.

v10 design:
  * Host lays out emb as embP16 [b, p, k, s]: input DMAs move 128
    partitions x multi-KB contiguous lines on the sync HWDGE ring
    (~420 B/ns). Batch 0 s-halved with per-half finish, batches 1-3
    k-halved, interleaved into the previous batch's dots stream.
  * Output PACKED [128, 4608] fp16 per batch, 3-4 contiguous DMAs;
    host unpacks + mirrors. All DMA on one ring, input first (FIFO
    priority).
  * PE warm-up dummies hold the HAM clock at 2.4GHz.
  * The proj_finish -> norms chain is pipelined in s-halves across the
    previous batch's dots hooks (i==3,4,5,6) so it never gates the PE.
  * Per i-tile: <=512 dots chunks into a deep 6-buf PSUM pool (shared
    with norm matmuls), drains (-2*d + n_i) on ACT (tiles 0-2) / DVE
    tensor_scalar (3-7), ONE wide fp16 rowrep add per tile on DVE
    (tiles 0-3) / GPSIMD (4-7). Last batch: drains ACT 0-4 / DVE 5-7,
    all adds on DVE (GPSIMD is slow and would stretch the tail).
"""

import numpy as np
from contextlib import ExitStack

import concourse.bass as bass
import concourse.bacc as bacc
import concourse.tile as tile
from concourse import mybir
from concourse.bass_utils import run_bass_kernel_spmd

B, S, D, R = 32, 1024, 1024, 128
NCORES = 8
BPC = B // NCORES
NDT = D // 128
NST = S // 128

F32 = mybir.dt.float32
F16 = mybir.dt.float16
IDENT = mybir.ActivationFunctionType.Identity
ADD = mybir.AluOpType.add
MULT = mybir.AluOpType.mult

WIDTHS = [S - 128 * i for i in range(NST)]
OFFS = [0]
for w in WIDTHS[:-1]:
    OFFS.append(OFFS[-1] + w)
TOT = OFFS[-1] + WIDTHS[-1]  # 4608

OUT_CUTS = [(1, 0, OFFS[2]), (3, OFFS[2], OFFS[4]), (7, OFFS[4], TOT)]
OUT_CUTS_LAST = [
    (1, 0, OFFS[2]), (3, OFFS[2], OFFS[4]),
    (5, OFFS[4], OFFS[6]), (7, OFFS[6], TOT),
]

N_WARM = 52


def build_nc():
    nc = bacc.Bacc("TRN2", target_bir_lowering=False, debug=False)

    embPd = nc.dram_tensor("embP16", [BPC, 128, NDT, S], F16, kind="ExternalInput")
    WTd = nc.dram_tensor("WT16", [128, D], F16, kind="ExternalInput")
    outPd = nc.dram_tensor("outP16", [BPC, 128, TOT], F16, kind="ExternalOutput")

    with tile.TileContext(nc) as tc, ExitStack() as ctx:
        constp = ctx.enter_context(tc.tile_pool(name="const", bufs=1))
        embT_p = ctx.enter_context(tc.tile_pool(name="embT", bufs=BPC))
        projT_p = ctx.enter_context(tc.tile_pool(name="projT", bufs=2))
        sq_p = ctx.enter_context(tc.tile_pool(name="sq", bufs=2))
        ncol_p = ctx.enter_context(tc.tile_pool(name="ncol", bufs=2))
        rowrep_p = ctx.enter_context(tc.tile_pool(name="rowrep", bufs=2))
        out_p = ctx.enter_context(tc.tile_pool(name="outsb", bufs=BPC))
        tmp_p = ctx.enter_context(tc.tile_pool(name="tmpsb", bufs=4))
        projps_p = ctx.enter_context(tc.tile_pool(name="projps", bufs=1, space="PSUM"))
        dotps_p = ctx.enter_context(tc.tile_pool(name="dotps", bufs=6, space="PSUM"))

        WT16 = constp.tile([128, D], F16, name="WT16")
        nc.sync.dma_start(out=WT16, in_=WTd.ap())

        ones16 = constp.tile([128, 128], F16, name="ones16")
        nc.vector.memset(ones16, 1.0)

        embTs = []
        for b in range(BPC):
            embT = embT_p.tile([128, NDT * S], F16, name="embT")
            embTs.append(embT)
            dst = embT.rearrange("p (k s) -> p k s", k=NDT)
            src = embPd.ap()[b]
            if b == 0:
                for h in range(2):
                    sl = slice(512 * h, 512 * (h + 1))
                    nc.sync.dma_start(out=dst[:, :, sl], in_=src[:, :, sl])
            else:
                for c in range(2):
                    ks = slice(4 * c, 4 * (c + 1))
                    nc.sync.dma_start(out=dst[:, ks, :], in_=src[:, ks, :])

        warm_ps = dotps_p.tile([128, 128], F32, tag="dp", name="warm_ps")
        for _ in range(N_WARM):
            nc.tensor.matmul(warm_ps, ones16, ones16, start=True, stop=True)

        def proj_alloc():
            projT = projT_p.tile([128, S], F16, name="projT")
            sq = sq_p.tile([128, S], F16, name="sq")
            pps = projps_p.tile([128, 1024], F32, name="projps")
            return projT, sq, pps

        def proj_finish(tiles, h):
            projT, sq, pps = tiles
            sl = slice(512 * h, 512 * (h + 1))
            nc.scalar.copy(projT[:, sl], pps[:, sl])
            nc.vector.tensor_mul(sq[:, sl], projT[:, sl], projT[:, sl])

        def proj_s_half(b, tiles, h):
            embT = embTs[b]
            _, _, pps = tiles
            for k in range(NDT):
                nc.tensor.matmul(
                    pps[:, 512 * h : 512 * (h + 1)],
                    WT16[:, 128 * k : 128 * (k + 1)],
                    embT[:, S * k + 512 * h : S * k + 512 * (h + 1)],
                    start=(k == 0),
                    stop=(k == NDT - 1),
                )

        def proj_k_chunk(b, tiles, k0, k1):
            embT = embTs[b]
            _, _, pps = tiles
            for k in range(k0, k1):
                for h in range(2):
                    nc.tensor.matmul(
                        pps[:, 512 * h : 512 * (h + 1)],
                        WT16[:, 128 * k : 128 * (k + 1)],
                        embT[:, S * k + 512 * h : S * k + 512 * (h + 1)],
                        start=(k == 0),
                        stop=(k == NDT - 1),
                    )

        def norms_alloc():
            ncol = ncol_p.tile([128, 2 * NST], F32, name="ncol")
            rowrep = rowrep_p.tile([128, S], F16, name="rowrep")
            return ncol, rowrep

        def norms_h(sq, ncol, rowrep, h):
            """ncol cols for tiles 4h..4h+3 and rowrep s-half h."""
            ncol_ps = dotps_p.tile([128, 8], F32, tag="dp", name="ncol_ps")
            for t in range(4):
                i = 4 * h + t
                nc.tensor.matmul(
                    ncol_ps[:, 2 * t : 2 * t + 2],
                    sq[:, 128 * i : 128 * (i + 1)],
                    ones16[:, 0:2],
                    start=True,
                    stop=True,
                )
            nc.vector.tensor_copy(ncol[:, 8 * h : 8 * h + 8], ncol_ps)
            rp = dotps_p.tile([128, 512], F32, tag="dp", name="rp_ps")
            nc.tensor.matmul(
                rp, ones16, sq[:, 512 * h : 512 * (h + 1)],
                start=True, stop=True,
            )
            nc.scalar.copy(rowrep[:, 512 * h : 512 * (h + 1)], rp)

        def dots_tile(b, i, outsb, projT, ncol, rowrep, last):
            j0 = 128 * i
            Wi = WIDTHS[i]
            off = OFFS[i]
            nb = ncol[:, 2 * i : 2 * i + 1]
            tmp = tmp_p.tile([128, 1024], F16, name="tmp")[:, 0:Wi]
            drain_act = i <= 4 if last else i <= 2
            pos = 0
            while pos < Wi:
                w = min(512, Wi - pos)
                d_ps = dotps_p.tile([128, w], F32, tag="dp", name="d_ps")
                nc.tensor.matmul(
                    d_ps,
                    projT[:, j0 : j0 + 128],
                    projT[:, j0 + pos : j0 + pos + w],
                    start=True,
                    stop=True,
                )
                tc_ = tmp[:, pos : pos + w]
                if drain_act:
                    nc.scalar.activation(tc_, d_ps, IDENT, bias=nb, scale=-2.0)
                else:
                    nc.vector.tensor_scalar(tc_, d_ps, -2.0, nb, MULT, ADD)
                pos += w
            o = outsb[:, off : off + Wi]
            rr = rowrep[:, j0:S]
            if i >= 4 and not last:
                nc.gpsimd.tensor_add(o, tmp, rr)
            else:
                nc.vector.tensor_add(o, tmp, rr)

        # ---- main pipeline ----
        tiles = proj_alloc()
        proj_s_half(0, tiles, 0)
        proj_finish(tiles, 0)
        proj_s_half(0, tiles, 1)
        proj_finish(tiles, 1)
        norms = norms_alloc()
        norms_h(tiles[1], norms[0], norms[1], 0)
        norms_h(tiles[1], norms[0], norms[1], 1)

        for b in range(BPC):
            last = b + 1 >= BPC
            projT, sq, _ = tiles
            ncol, rowrep = norms
            outsb = out_p.tile([128, TOT], F16, name="outsb")
            cuts = OUT_CUTS_LAST if last else OUT_CUTS
            cut = 0
            for i in range(NST):
                dots_tile(b, i, outsb, projT, ncol, rowrep, last)
                if cut < len(cuts) and cuts[cut][0] == i:
                    _, c0, c1 = cuts[cut]
                    nc.sync.dma_start(
                        out=outPd.ap()[b, :, c0:c1], in_=outsb[:, c0:c1]
                    )
                    cut += 1
                if not last:
                    if i == 1:
                        tiles_n = proj_alloc()
                        proj_k_chunk(b + 1, tiles_n, 0, 4)
                    elif i == 3:
                        proj_k_chunk(b + 1, tiles_n, 4, NDT)
                        proj_finish(tiles_n, 0)
                    elif i == 4:
                        proj_finish(tiles_n, 1)
                        norms_n = norms_alloc()
                    elif i == 5:
                        norms_h(tiles_n[1], norms_n[0], norms_n[1], 0)
                    elif i == 6:
                        norms_h(tiles_n[1], norms_n[0], norms_n[1], 1)
            if not last:
                tiles = tiles_n
                norms = norms_n

    nc.finalize()
    return nc


_NC_CACHE = None


def _get_nc():
    global _NC_CACHE
    if _NC_CACHE is None:
        _NC_CACHE = build_nc()
    return _NC_CACHE


def _host_wt16(W):
    Wf = np.asarray(W, dtype=np.float32)
    wt = Wf.T.reshape(NDT, 128, 128).transpose(1, 0, 2).reshape(128, D)
    return np.ascontiguousarray(wt).astype(np.float16)


def _host_embp(emb16_core):
    return np.ascontiguousarray(
        emb16_core.reshape(BPC, S, NDT, 128).transpose(0, 3, 2, 1)
    )


def run(embeddings_batch, W, trace=False, tmpdir=None):
    nc = _get_nc()
    emb16 = np.asarray(embeddings_batch, dtype=np.float32).astype(np.float16)
    wt16 = _host_wt16(W)
    in_maps = [
        {
            "embP16": _host_embp(emb16[c * BPC : (c + 1) * BPC]),
            "WT16": wt16,
        }
        for c in range(NCORES)
    ]
    res = run_bass_kernel_spmd(
        nc, in_maps, core_ids=list(range(NCORES)), trace=trace, tmpdir=tmpdir
    )
    full = np.empty((B, S, S), dtype=np.float16)
    for c in range(NCORES):
        P = res.results[c]["outP16"]
        for b in range(BPC):
            g = c * BPC + b
            for i in range(NST):
                full[g, 128 * i : 128 * (i + 1), 128 * i : S] = P[
                    b, :, OFFS[i] : OFFS[i] + WIDTHS[i]
                ]
    NB = NST
    M = full.reshape(B, NB, 128, NB, 128)
    iu = np.triu_indices(NB, 1)
    M[:, iu[1], :, iu[0], :] = M[:, iu[0], :, iu[1], :].swapaxes(-1, -2)
    return full.astype(np.float32), res


def kernel(embeddings_batch, W):
    full, _ = run(embeddings_batch, W, trace=False)
    return full
